# revision 11
# baseline (speedup 1.0000x reference)
"""Trainium2 Bass kernel for nn_ExportableGENConv (GENConv message passing +
channelwise softmax aggregation + MLP with global-batch BatchNorm), sharded
across 8 NeuronCores.

Contract: kernel(**inputs) takes the FULL inputs of reference.setup_inputs()
and returns the FULL [32768, 64] float32 output.

Sharding: nodes (each with K=32 contiguous incoming edge slots) are split
across 8 cores. Per-edge source features x[src] are materialized host-side
during staging (the halo exchange) in bf16, the per-edge message + per-node
softmax + MLP run on device. Global BatchNorm statistics are combined on host
between two NEFF launches: phase 1 produces h1 = (aggregated+x) @ W1 plus
per-core sum/sumsq, phase 2 applies the batch-norm affine + ReLU + W2.

Math (per node i, channel h, over valid slots k):
  t = x[src] + ea @ W_edge            (invalid slots: staged x = -1e9 -> t << 0)
  reference: m = relu(t) + 1e-7; softmax over k of m; res = sum m*alpha.
  With r = relu(t):  res = (sum_k r*e^r) / (sum_k e^r + 1e-16) + 1e-7
  Device: r = relu(t) (fp16); P = exp(r) (bf16; invalid slots contribute
  exactly 1.0, removed via a host-staged per-node count); Pm = r*P (bf16).

Device layout ("k-layout"): node-tile = 32 nodes = 1024 edge slots laid out as
partition p = (node%32)*4 + (k%4), free = (b=k//4 in [0,8), h). Tiles are
processed in chunks of 2 (one [128,1024] 2-bank PSUM tile per chunk) so the
scalar/vector elementwise passes amortize their fixed access latency. The
softmax reduction is a PE matmul with a block-diagonal ones stationary over
partition blocks of 4 (stage 1, bf16 PSUM out, col-offset-stacked over 4
node-tiles) + a DVE reduce over b (stage 2, innermost-strided AP). Engine
balance: relu alternates scalar(3):vector(1); Pm alternates vector/gpsimd;
the per-node combine + transpose + h1 matmul + BN-stats epilogue is pipelined
into the loop in batches of 4 node-groups (emitted one batch late so the PE
never head-of-line blocks on the DVE combine chain).
"""

import numpy as np
from contextlib import ExitStack

import concourse.bass as bass
import concourse.tile as tile
from concourse import mybir
from concourse.bass_utils import run_bass_kernel_spmd

try:
    from ml_dtypes import bfloat16 as np_bf16
except ImportError:  # ml_dtypes ships with jax; fall back just in case
    import jax.numpy as _jnp

    np_bf16 = _jnp.bfloat16

# ---------------------------------------------------------------- constants
N, K, H, ED = 32768, 32, 64, 32
E = N * K
NCORES = 8
NPC = N // NCORES            # nodes per core = 4096
NT = NPC // 32               # node-tiles per core = 128
NCH = NT // 2                # 2-tile chunks per core = 64
NEG_BIG = -1.0e9

_compiled = {}


# ------------------------------------------------------- multi-wait legalizer
def _legalize_multiwaits(nc):
    """This walrus build accepts only ONE sync wait per instruction; move the
    excess onto injected same-engine drain carriers placed immediately before
    the instruction (semantics-preserving: the engine stalls there instead)."""
    n_injected = 0
    for fn in nc.m.functions:
        for blk in fn.blocks:
            bb = blk if hasattr(blk, "instructions") else blk.bb
            insts = list(bb.instructions)
            out = []
            for inst in insts:
                si = inst.sync_info
                if si is not None and si.on_wait and len(si.on_wait) > 1:
                    waits = list(si.on_wait)
                    for w in waits[:-1]:
                        nop = mybir.InstDrain(
                            name=f"waitfix-{nc.next_id()}", ins=[], outs=[]
                        )
                        nop.engine = inst.engine
                        nop.sync_info = mybir.SyncInfo(on_wait=[w], on_update=[])
                        nc.register_instruction(nop, overwrite=True)
                        out.append(nop)
                        n_injected += 1
                    inst.sync_info = mybir.SyncInfo(
                        on_wait=[waits[-1]], on_update=list(si.on_update or [])
                    )
                out.append(inst)
            if len(out) != len(insts):
                bb.instructions = out
    return n_injected


# ------------------------------------------------------------ phase-1 kernel
def _build_phase1(ntiles=NT, use_gpsimd=True, use_fast_recip=False):
    fp32 = mybir.dt.float32
    bf16 = mybir.dt.bfloat16
    fp16 = mybir.dt.float16
    Act = mybir.ActivationFunctionType
    Alu = mybir.AluOpType
    npc = ntiles * 32
    ngrp = ntiles // 4
    nchunk = ntiles // 2
    nbatch = ngrp // 4           # epilogue batches (4 node-groups each)
    nc = bass.Bass()

    xae_d = nc.declare_dram_parameter("xae", [128, nchunk * 1536], bf16,
                                      isOutput=False)
    wbd_d = nc.declare_dram_parameter("wbd", [128, 256], bf16, isOutput=False)
    bd_d = nc.declare_dram_parameter("bd", [128, 32], bf16, isOutput=False)
    id16_d = nc.declare_dram_parameter("id16", [128, 128], bf16, isOutput=False)
    id32_d = nc.declare_dram_parameter("id32", [128, 128], fp32, isOutput=False)
    corr_d = nc.declare_dram_parameter("corr", [128, ngrp * 64], fp32, isOutput=False)
    xres_d = nc.declare_dram_parameter("xres", [128, ngrp * 64], fp32, isOutput=False)
    w1_d = nc.declare_dram_parameter("w1", [64, 128], fp32, isOutput=False)
    h1_d = nc.declare_dram_parameter("h1", [128, npc], fp32, isOutput=True)
    st_d = nc.declare_dram_parameter("stats", [128, 2], fp32, isOutput=True)

    with tile.TileContext(nc) as tc, ExitStack() as ctx:
        const = ctx.enter_context(tc.tile_pool(name="const", bufs=1))
        sb = ctx.enter_context(tc.tile_pool(name="sb", bufs=1))
        xae_p = ctx.enter_context(tc.tile_pool(name="xae", bufs=4))
        r_p = ctx.enter_context(tc.tile_pool(name="rr", bufs=4))
        pp_p = ctx.enter_context(tc.tile_pool(name="pp", bufs=3))
        pm_p = ctx.enter_context(tc.tile_pool(name="pm", bufs=3))
        epi_p = ctx.enter_context(tc.tile_pool(name="epi", bufs=2))
        h1s_p = ctx.enter_context(tc.tile_pool(name="h1s", bufs=2))
        sq_p = ctx.enter_context(tc.tile_pool(name="sq", bufs=2))
        ps_t = ctx.enter_context(tc.tile_pool(name="ps_t", bufs=3, space="PSUM"))
        ps_s = ctx.enter_context(tc.tile_pool(name="ps_s", bufs=1, space="PSUM"))
        ps_u = ctx.enter_context(tc.tile_pool(name="ps_u", bufs=1, space="PSUM"))

        wbd_t = const.tile([128, 256], bf16)
        nc.sync.dma_start(wbd_t[:], wbd_d[:])
        bd_t = const.tile([128, 32], bf16)
        nc.sync.dma_start(bd_t[:], bd_d[:])
        id16_t = const.tile([128, 128], bf16)
        nc.sync.dma_start(id16_t[:], id16_d[:])
        id32_t = const.tile([128, 128], fp32)
        nc.sync.dma_start(id32_t[:], id32_d[:])
        corr_t = const.tile([128, ngrp * 64], fp32)
        nc.sync.dma_start(corr_t[:], corr_d[:])
        xres_t = const.tile([128, ngrp * 64], fp32)
        nc.sync.dma_start(xres_t[:], xres_d[:])
        w1_t = const.tile([64, 128], fp32)
        nc.sync.dma_start(w1_t[:], w1_d[:])

        S2_all = sb.tile([128, ngrp * 64], fp32)
        T2_all = sb.tile([128, ngrp * 64], fp32)
        outT = sb.tile([64, npc], fp32)
        s1p = sb.tile([128, nbatch], fp32)
        s2p = sb.tile([128, nbatch], fp32)

        # prologue: make PE observe const DMA sems via tiny touch matmuls
        pro_ps = ps_s.tile([128, 512], fp32, tag="ps_s")
        nc.tensor.matmul(pro_ps[0:8, 0:8], id16_t[:, 0:8], id16_t[:, 0:8],
                         start=True, stop=True, skip_group_check=True)
        nc.tensor.matmul(pro_ps[0:8, 0:8], wbd_t[:, 0:8], wbd_t[:, 0:8],
                         start=True, stop=True, skip_group_check=True)
        nc.tensor.matmul(pro_ps[0:8, 0:8], bd_t[:, 0:8], bd_t[:, 0:8],
                         start=True, stop=True, skip_group_check=True)
        nc.tensor.matmul(pro_ps[0:8, 0:8], id32_t[:, 0:8], id32_t[:, 0:8],
                         start=True, stop=True, skip_group_check=True)
        nc.tensor.matmul(pro_ps[0:8, 0:8], w1_t[:, 0:8], w1_t[:, 0:8],
                         start=True, stop=True, skip_group_check=True)

        pending = {}             # batch id -> res tile awaiting PE epilogue

        def emit_batch_dve(B):
            """Per-node combine for node-groups 4B..4B+3 (DVE only)."""
            sl = slice(B * 256, (B + 1) * 256)
            den = epi_p.tile([128, 256], fp32, tag="den")
            nc.vector.tensor_tensor(out=den[:], in0=S2_all[:, sl],
                                    in1=corr_t[:, sl], op=Alu.subtract)
            rec = epi_p.tile([128, 256], fp32, tag="rec")
            if use_fast_recip:
                nc.vector.reciprocal_approx_fast(rec[:], den[:])
            else:
                nc.vector.reciprocal(rec[:], den[:])
            resb = epi_p.tile([128, 256], fp32, tag="resb")
            nc.vector.tensor_tensor(out=resb[:], in0=T2_all[:, sl], in1=rec[:],
                                    op=Alu.mult)
            nc.vector.scalar_tensor_tensor(
                resb[:], resb[:], 1e-7, xres_t[:, sl], op0=Alu.add, op1=Alu.add)
            pending[B] = resb

        def emit_batch_pe(B):
            """Transpose + h1 matmul + stats for a completed combine batch."""
            resb = pending.pop(B)
            trh = ps_t.tile([128, 1024], fp32, tag="ps_t")
            for q in range(4):
                nc.tensor.matmul(trh[0:64, 128 * q:128 * (q + 1)],
                                 resb[:, 64 * q:64 * (q + 1)], id32_t[:],
                                 is_transpose=True, skip_group_check=True)
            nc.vector.tensor_copy(outT[:, 512 * B:512 * (B + 1)],
                                  trh[0:64, 0:512])
            nc.tensor.matmul(trh[:, 512:1024], w1_t[:],
                             outT[:, 512 * B:512 * (B + 1)],
                             start=True, stop=True, skip_group_check=True)
            h1sl = h1s_p.tile([128, 512], fp32, tag="h1s")
            nc.scalar.copy(h1sl[:], trh[:, 512:1024])
            nc.sync.dma_start(h1_d[:, 512 * B:512 * (B + 1)], h1sl[:])
            nc.vector.tensor_reduce(s1p[:, B:B + 1], h1sl[:],
                                    axis=mybir.AxisListType.X, op=Alu.add)
            sq = sq_p.tile([128, 512], fp32, tag="sq")
            nc.vector.scalar_tensor_tensor(
                sq[:], h1sl[:], 0.0, h1sl[:], op0=Alu.add, op1=Alu.mult,
                accum_out=s2p[:, B:B + 1])

        # ---- edge phase (chunks of 2 node-tiles = 2048 edges)
        s1_pair = [None, None]
        for j in range(nchunk):
            if j % 8 == 0 and j > 0:
                emit_batch_pe(j // 8 - 1)

            xae_t = xae_p.tile([128, 1536], bf16, tag="xae")
            nc.sync.dma_start(xae_t[:], xae_d[:, j * 1536:(j + 1) * 1536])

            t_ps = ps_t.tile([128, 1024], fp32, tag="ps_t")
            for i in range(2):
                nc.tensor.matmul(t_ps[:, 512 * i:512 * (i + 1)], id16_t[:],
                                 xae_t[:, 512 * i:512 * (i + 1)],
                                 start=True, stop=False)
                for g in range(2):
                    nc.tensor.matmul(
                        t_ps[:, 512 * i + 256 * g:512 * i + 256 * (g + 1)],
                        xae_t[:, 1024 + 256 * i + 128 * g:
                              1024 + 256 * i + 128 * (g + 1)],
                        wbd_t[:], start=False, stop=(g == 1))

            # r = relu(t): 3 of 4 chunks on scalar, 1 of 4 on vector
            r_t = r_p.tile([128, 1024], fp16, tag="rr")
            if j % 4 == 3:
                nc.vector.tensor_scalar_max(r_t[:], t_ps[:], 0.0)
            else:
                nc.scalar.activation(r_t[:], t_ps[:], Act.Relu)
            # P = exp(r) (scalar); Pm = r * P (vector/gpsimd alternating)
            P_t = pp_p.tile([128, 1024], bf16, tag="pp")
            nc.scalar.activation(P_t[:], r_t[:], Act.Exp)
            Pm_t = pm_p.tile([128, 1024], bf16, tag="pm")
            if use_gpsimd and j % 2 == 1:
                nc.gpsimd.tensor_tensor(out=Pm_t[:], in0=r_t[:], in1=P_t[:],
                                        op=Alu.mult)
            else:
                nc.vector.tensor_tensor(out=Pm_t[:], in0=r_t[:], in1=P_t[:],
                                        op=Alu.mult)

            for i in range(2):
                T = 2 * j + i
                c = T % 4
                if c == 0:
                    s1_pair = [
                        ps_s.tile([128, 512], fp32, tag="ps_s", name=f"S1_{T}"),
                        ps_u.tile([128, 512], fp32, tag="ps_u", name=f"T1_{T}"),
                    ]
                S1_ps, T1_ps = s1_pair
                nc.tensor.matmul(S1_ps[32 * c:32 * c + 32, :], bd_t[:],
                                 P_t[:, 512 * i:512 * (i + 1)],
                                 start=True, stop=True, tile_position=(0, 32 * c),
                                 skip_group_check=True)
                nc.tensor.matmul(T1_ps[32 * c:32 * c + 32, :], bd_t[:],
                                 Pm_t[:, 512 * i:512 * (i + 1)],
                                 start=True, stop=True, tile_position=(0, 32 * c),
                                 skip_group_check=True)

                if c == 3:
                    G = T // 4
                    nc.vector.tensor_reduce(
                        S2_all[:, G * 64:(G + 1) * 64],
                        S1_ps[:].rearrange("p (b h) -> p h b", h=H),
                        axis=mybir.AxisListType.X, op=Alu.add)
                    nc.vector.tensor_reduce(
                        T2_all[:, G * 64:(G + 1) * 64],
                        T1_ps[:].rearrange("p (b h) -> p h b", h=H),
                        axis=mybir.AxisListType.X, op=Alu.add)
                    if G % 4 == 3:
                        emit_batch_dve(G // 4)

        emit_batch_pe(nbatch - 1)

        # ---- finalize BN stats
        s1 = sb.tile([128, 1], fp32)
        nc.vector.tensor_reduce(s1[:], s1p[:], axis=mybir.AxisListType.X,
                                op=Alu.add)
        s2 = sb.tile([128, 1], fp32)
        nc.vector.tensor_reduce(s2[:], s2p[:], axis=mybir.AxisListType.X,
                                op=Alu.add)
        stats = sb.tile([128, 2], fp32)
        nc.vector.tensor_copy(stats[:, 0:1], s1[:])
        nc.vector.tensor_copy(stats[:, 1:2], s2[:])
        nc.scalar.dma_start(st_d[:], stats[:])

    _legalize_multiwaits(nc)
    return nc


# ------------------------------------------------------------ phase-2 kernel
def _build_phase2(ntiles=NT):
    fp32 = mybir.dt.float32
    Act = mybir.ActivationFunctionType
    npc = ntiles * 32
    nslice = npc // 512
    nc = bass.Bass()

    h1_d = nc.declare_dram_parameter("h1", [128, npc], fp32, isOutput=False)
    ss_d = nc.declare_dram_parameter("ss", [128, 2], fp32, isOutput=False)
    w2_d = nc.declare_dram_parameter("w2", [128, 64], fp32, isOutput=False)
    out_d = nc.declare_dram_parameter("out", [64, npc], fp32, isOutput=True)

    with tile.TileContext(nc) as tc, ExitStack() as ctx:
        const = ctx.enter_context(tc.tile_pool(name="const", bufs=1))
        h1_p = ctx.enter_context(tc.tile_pool(name="h1p", bufs=3))
        h2_p = ctx.enter_context(tc.tile_pool(name="h2p", bufs=3))
        osl_p = ctx.enter_context(tc.tile_pool(name="osl", bufs=3))
        ps = ctx.enter_context(tc.tile_pool(name="ps", bufs=3, space="PSUM"))

        ss_t = const.tile([128, 2], fp32)
        nc.sync.dma_start(ss_t[:], ss_d[:])
        w2_t = const.tile([128, 64], fp32)
        nc.sync.dma_start(w2_t[:], w2_d[:])

        pro_ps = ps.tile([128, 512], fp32, tag="ps")
        nc.tensor.matmul(pro_ps[0:8, 0:8], w2_t[:, 0:8], w2_t[:, 0:8],
                         start=True, stop=True, skip_group_check=True)

        for j in range(nslice):
            h1sl = h1_p.tile([128, 512], fp32, tag="h1p")
            nc.sync.dma_start(h1sl[:], h1_d[:, j * 512:(j + 1) * 512])
            h2sl = h2_p.tile([128, 512], fp32, tag="h2p")
            nc.scalar.activation(h2sl[:], h1sl[:], Act.Relu, bias=ss_t[:, 1:2],
                                 scale=ss_t[:, 0:1])
            o_ps = ps.tile([128, 512], fp32, tag="ps")
            nc.tensor.matmul(o_ps[0:64, :], w2_t[:], h2sl[:],
                             start=True, stop=True, skip_group_check=True)
            oslice = osl_p.tile([64, 512], fp32, tag="osl")
            nc.scalar.copy(oslice[:], o_ps[0:64, :])
            nc.scalar.dma_start(out_d[:, j * 512:(j + 1) * 512], oslice[:])

    _legalize_multiwaits(nc)
    return nc


# -------------------------------------------------------------- host staging
def _stage_core(x_c, xs_slot_c, ea_slot_c, ninv_c, ntiles=NT):
    """xs_slot_c: [npc, K, H] f32 (x[src], invalid slots = NEG_BIG)
    ea_slot_c: [npc, K, ED] f32;  ninv_c: [npc] f32.

    Returns (xae bf16 [128, nchunk*1536], corr f32, xres f32)."""
    a = xs_slot_c.reshape(ntiles, 32, 8, 4, H)          # [T, m, b, j, h]
    xs_dev = np.ascontiguousarray(
        a.transpose(1, 3, 0, 2, 4)).reshape(128, ntiles * 512)

    b = ea_slot_c.reshape(ntiles, 32, 8, 4, ED)         # [T, m, b, j, d]
    ea4 = np.ascontiguousarray(
        b.transpose(2, 4, 0, 1, 3)                      # [b, d, T, m, j]
        .reshape(2, 4, ED, ntiles, 128)                 # [g, r, d, T, e']
        .transpose(1, 2, 3, 0, 4)                       # [r, d, T, g, e']
    ).reshape(128, ntiles * 256)

    # fuse xs + ea into one per-chunk DMA block:
    # chunk j: [xs(2j) 512 | xs(2j+1) 512 | ea(2j) 256 | ea(2j+1) 256]
    nch = ntiles // 2
    xs3 = xs_dev.reshape(128, nch, 1024)
    ea3 = ea4.reshape(128, nch, 512)
    xae = np.concatenate([xs3, ea3], axis=2).astype(np_bf16)
    xae = np.ascontiguousarray(xae).reshape(128, nch * 1536)

    # node n = 128*G + p'  (p' = 32*(T%4) + node%32)
    corr = (ninv_c.astype(np.float32) - 1e-16)[:, None] * np.ones((1, H), np.float32)
    corr_dev = np.ascontiguousarray(
        corr.reshape(ntiles // 4, 128, H).transpose(1, 0, 2)).reshape(128, -1)
    xres_dev = np.ascontiguousarray(
        x_c.reshape(ntiles // 4, 128, H).transpose(1, 0, 2)).reshape(128, -1)
    return xae, corr_dev, xres_dev


def _consts(W_edge):
    Wbd = np.zeros((128, 256), np.float32)
    for r in range(4):
        Wbd[32 * r:32 * r + 32, 64 * r:64 * r + 64] = W_edge
    BD = np.zeros((128, 32), np.float32)
    for m in range(32):
        BD[4 * m:4 * m + 4, m] = 1.0
    ident = np.eye(128, dtype=np.float32)
    return Wbd, BD, ident


def build_in_maps(x, edge_index, edge_attr, nbr, W_edge, W1):
    """Stage the full inputs into per-core phase-1 input maps."""
    x = np.ascontiguousarray(np.asarray(x, dtype=np.float32))
    edge_attr = np.ascontiguousarray(np.asarray(edge_attr, dtype=np.float32))
    W_edge = np.ascontiguousarray(np.asarray(W_edge, dtype=np.float32))
    W1 = np.ascontiguousarray(np.asarray(W1, dtype=np.float32))

    src = np.asarray(edge_index[0], dtype=np.int64)
    nbr = np.asarray(nbr)
    valid = nbr >= 0                                    # [N, K]
    expect = np.arange(E, dtype=np.int64).reshape(N, K)
    assert np.array_equal(np.where(valid, nbr, expect), expect), \
        "kernel assumes nbr[i,k] == i*K+k on valid slots"

    src_slot = src.reshape(N, K)
    xs_slot = x[src_slot]                               # host halo: [N, K, H]
    xs_slot[~valid] = NEG_BIG
    ninv = (~valid).sum(axis=1).astype(np.float32)      # [N]
    ea_slot = edge_attr.reshape(N, K, ED)

    Wbd, BD, ident = _consts(W_edge)
    Wbd16 = Wbd.astype(np_bf16)
    BD16 = BD.astype(np_bf16)
    id16 = ident.astype(np_bf16)

    in_maps = []
    for core in range(NCORES):
        sl = slice(core * NPC, (core + 1) * NPC)
        xae, corr_dev, xres_dev = _stage_core(
            x[sl], xs_slot[sl], ea_slot[sl], ninv[sl])
        in_maps.append({
            "xae": xae, "wbd": Wbd16, "bd": BD16, "id16": id16,
            "id32": ident, "corr": corr_dev, "xres": xres_dev, "w1": W1,
        })
    return in_maps


def kernel(x, edge_index, edge_attr, nbr, W_edge, W1, gamma, beta, W2):
    gamma = np.asarray(gamma, dtype=np.float32)
    beta = np.asarray(beta, dtype=np.float32)
    W2 = np.ascontiguousarray(np.asarray(W2, dtype=np.float32))

    in_maps = build_in_maps(x, edge_index, edge_attr, nbr, W_edge, W1)

    if "p1" not in _compiled:
        _compiled["p1"] = _build_phase1(NT)
        _compiled["p2"] = _build_phase2(NT)

    res1 = run_bass_kernel_spmd(_compiled["p1"], in_maps,
                                core_ids=list(range(NCORES)))

    # host: combine BN stats (tiny 128-vector arithmetic), build scale/shift
    s1 = np.zeros(2 * H, np.float64)
    s2 = np.zeros(2 * H, np.float64)
    for core in range(NCORES):
        st = res1.results[core]["stats"].astype(np.float64)
        s1 += st[:, 0]
        s2 += st[:, 1]
    mean = (s1 / N).astype(np.float32)
    var = (s2 / N).astype(np.float32) - mean * mean
    scale = gamma / np.sqrt(var + 1e-5)
    shift = beta - mean * scale
    ss = np.stack([scale, shift], axis=1).astype(np.float32)  # [128, 2]

    in_maps2 = [{"h1": res1.results[core]["h1"], "ss": ss, "w2": W2}
                for core in range(NCORES)]
    res2 = run_bass_kernel_spmd(_compiled["p2"], in_maps2,
                                core_ids=list(range(NCORES)))

    out = np.empty((N, H), np.float32)
    for core in range(NCORES):
        out[core * NPC:(core + 1) * NPC] = res2.results[core]["out"].T
    return out


# revision 22
# speedup vs baseline: 1.2123x; 1.2123x over previous
"""Trainium2 Bass kernel for nn_ExportableGENConv (GENConv message passing +
channelwise softmax aggregation + MLP with global-batch BatchNorm), sharded
across 8 NeuronCores.

Contract: kernel(**inputs) takes the FULL inputs of reference.setup_inputs()
and returns the FULL [32768, 64] float32 output.

Sharding: nodes (each with K=32 contiguous incoming edge slots) are split
across 8 cores. Per-edge source features x[src] are materialized host-side
during staging (the halo exchange) in bf16, the per-edge message + per-node
softmax + MLP run on device. Global BatchNorm statistics are combined on host
between two NEFF launches: phase 1 produces h1 = (aggregated+x) @ W1 plus
per-core sum/sumsq, phase 2 applies the batch-norm affine + ReLU + W2.

Math (per node i, channel h, over valid slots k):
  t = x[src] + ea @ W_edge            (invalid slots: staged x = -1e9 -> t << 0)
  reference: m = relu(t) + 1e-7; softmax over k of m; res = sum m*alpha.
  With r = relu(t):  res = (sum_k r*e^r) / (sum_k e^r + 1e-16) + 1e-7
  Device: r = relu(t) (fp16); P = exp(r) (bf16; invalid slots contribute
  exactly 1.0, removed via a host-staged per-node count); Pm = r*P (bf16).

Device layout ("k-layout"): node-tile = 32 nodes = 1024 edge slots laid out as
partition p = (node%32)*4 + (k%4), free = (b=k//4 in [0,8), h). Tiles are
processed in chunks of 2 (one [128,1024] 2-bank PSUM tile per chunk) so the
scalar/vector elementwise passes amortize their fixed access latency. The
softmax reduction is a PE matmul with a block-diagonal ones stationary over
partition blocks of 4 (stage 1, bf16 PSUM out, col-offset-stacked over 4
node-tiles) + a DVE reduce over b (stage 2, innermost-strided AP). Engine
balance: relu alternates scalar(3):vector(1); Pm alternates vector/gpsimd;
the per-node combine + transpose + h1 matmul + BN-stats epilogue is pipelined
into the loop in batches of 4 node-groups (emitted one batch late so the PE
never head-of-line blocks on the DVE combine chain).
"""

import numpy as np
from contextlib import ExitStack

import concourse.bass as bass
import concourse.tile as tile
from concourse import mybir
from concourse.bass_utils import run_bass_kernel_spmd

try:
    from ml_dtypes import bfloat16 as np_bf16
except ImportError:  # ml_dtypes ships with jax; fall back just in case
    import jax.numpy as _jnp

    np_bf16 = _jnp.bfloat16

# ---------------------------------------------------------------- constants
N, K, H, ED = 32768, 32, 64, 32
E = N * K
NCORES = 8
NPC = N // NCORES            # nodes per core = 4096
NT = NPC // 32               # node-tiles per core = 128
NCH = NT // 2                # 2-tile chunks per core = 64
NEG_BIG = -1.0e9

_compiled = {}


# ------------------------------------------------------- multi-wait legalizer
def _legalize_multiwaits(nc):
    """This walrus build accepts only ONE sync wait per instruction; move the
    excess onto injected same-engine drain carriers placed immediately before
    the instruction (semantics-preserving: the engine stalls there instead)."""
    n_injected = 0
    for fn in nc.m.functions:
        for blk in fn.blocks:
            bb = blk if hasattr(blk, "instructions") else blk.bb
            insts = list(bb.instructions)
            out = []
            for inst in insts:
                si = inst.sync_info
                if si is not None and si.on_wait and len(si.on_wait) > 1:
                    waits = list(si.on_wait)
                    for w in waits[:-1]:
                        nop = mybir.InstDrain(
                            name=f"waitfix-{nc.next_id()}", ins=[], outs=[]
                        )
                        nop.engine = inst.engine
                        nop.sync_info = mybir.SyncInfo(on_wait=[w], on_update=[])
                        nc.register_instruction(nop, overwrite=True)
                        out.append(nop)
                        n_injected += 1
                    inst.sync_info = mybir.SyncInfo(
                        on_wait=[waits[-1]], on_update=list(si.on_update or [])
                    )
                out.append(inst)
            if len(out) != len(insts):
                bb.instructions = out
    return n_injected


# ------------------------------------------------------------ phase-1 kernel
def _build_phase1(ntiles=NT, use_gpsimd=False, use_fast_recip=False):
    fp32 = mybir.dt.float32
    bf16 = mybir.dt.bfloat16
    fp16 = mybir.dt.float16
    Act = mybir.ActivationFunctionType
    Alu = mybir.AluOpType
    npc = ntiles * 32
    ngrp = ntiles // 4
    nchunk = ntiles // 2
    nbatch = ngrp // 4           # epilogue batches (4 node-groups each)
    nc = bass.Bass()

    xae_d = nc.declare_dram_parameter("xae", [128, nchunk * 1536], bf16,
                                      isOutput=False)
    wbd_d = nc.declare_dram_parameter("wbd", [128, 256], bf16, isOutput=False)
    bd_d = nc.declare_dram_parameter("bd", [128, 32], bf16, isOutput=False)
    id16_d = nc.declare_dram_parameter("id16", [128, 128], bf16, isOutput=False)
    id32_d = nc.declare_dram_parameter("id32", [128, 128], fp32, isOutput=False)
    corr_d = nc.declare_dram_parameter("corr", [128, ngrp * 64], fp32, isOutput=False)
    xres_d = nc.declare_dram_parameter("xres", [128, ngrp * 64], fp32, isOutput=False)
    w1_d = nc.declare_dram_parameter("w1", [64, 128], fp32, isOutput=False)
    h1_d = nc.declare_dram_parameter("h1", [128, npc], fp32, isOutput=True)
    st_d = nc.declare_dram_parameter("stats", [128, 2], fp32, isOutput=True)

    with tile.TileContext(nc) as tc, ExitStack() as ctx:
        const = ctx.enter_context(tc.tile_pool(name="const", bufs=1))
        sb = ctx.enter_context(tc.tile_pool(name="sb", bufs=1))
        xae_p = ctx.enter_context(tc.tile_pool(name="xae", bufs=4))
        r_p = ctx.enter_context(tc.tile_pool(name="rr", bufs=4))
        pp_p = ctx.enter_context(tc.tile_pool(name="pp", bufs=4))
        pm_p = ctx.enter_context(tc.tile_pool(name="pm", bufs=4))
        epi_p = ctx.enter_context(tc.tile_pool(name="epi", bufs=2))
        h1s_p = ctx.enter_context(tc.tile_pool(name="h1s", bufs=2))
        sq_p = ctx.enter_context(tc.tile_pool(name="sq", bufs=2))
        ps_t = ctx.enter_context(tc.tile_pool(name="ps_t", bufs=2, space="PSUM"))
        ps_s = ctx.enter_context(tc.tile_pool(name="ps_s", bufs=2, space="PSUM"))
        ps_u = ctx.enter_context(tc.tile_pool(name="ps_u", bufs=2, space="PSUM"))

        wbd_t = const.tile([128, 256], bf16)
        nc.sync.dma_start(wbd_t[:], wbd_d[:])
        bd_t = const.tile([128, 32], bf16)
        nc.sync.dma_start(bd_t[:], bd_d[:])
        id16_t = const.tile([128, 128], bf16)
        nc.sync.dma_start(id16_t[:], id16_d[:])
        id32_t = const.tile([128, 128], fp32)
        nc.sync.dma_start(id32_t[:], id32_d[:])
        corr_t = const.tile([128, ngrp * 64], fp32)
        nc.sync.dma_start(corr_t[:], corr_d[:])
        xres_t = const.tile([128, ngrp * 64], fp32)
        nc.sync.dma_start(xres_t[:], xres_d[:])
        w1_t = const.tile([64, 128], fp32)
        nc.sync.dma_start(w1_t[:], w1_d[:])

        S2_all = sb.tile([128, ngrp * 64], fp32)
        T2_all = sb.tile([128, ngrp * 64], fp32)
        outT = sb.tile([64, npc], fp32)
        s1p = sb.tile([128, nbatch], fp32)
        s2p = sb.tile([128, nbatch], fp32)

        # prologue: make PE observe const DMA sems via tiny touch matmuls
        pro_ps = ps_s.tile([128, 512], fp32, tag="ps_s")
        nc.tensor.matmul(pro_ps[0:8, 0:8], id16_t[:, 0:8], id16_t[:, 0:8],
                         start=True, stop=True, skip_group_check=True)
        nc.tensor.matmul(pro_ps[0:8, 0:8], wbd_t[:, 0:8], wbd_t[:, 0:8],
                         start=True, stop=True, skip_group_check=True)
        nc.tensor.matmul(pro_ps[0:8, 0:8], bd_t[:, 0:8], bd_t[:, 0:8],
                         start=True, stop=True, skip_group_check=True)
        nc.tensor.matmul(pro_ps[0:8, 0:8], id32_t[:, 0:8], id32_t[:, 0:8],
                         start=True, stop=True, skip_group_check=True)
        nc.tensor.matmul(pro_ps[0:8, 0:8], w1_t[:, 0:8], w1_t[:, 0:8],
                         start=True, stop=True, skip_group_check=True)

        pending = {}             # batch id -> res tile awaiting PE epilogue

        def emit_batch_dve(B):
            """Per-node combine for node-groups 4B..4B+3 (gpsimd + DVE recip)."""
            sl = slice(B * 256, (B + 1) * 256)
            eng = nc.gpsimd if use_gpsimd else nc.vector
            den = epi_p.tile([128, 256], fp32, tag="den")
            eng.tensor_tensor(out=den[:], in0=S2_all[:, sl],
                              in1=corr_t[:, sl], op=Alu.subtract)
            rec = epi_p.tile([128, 256], fp32, tag="rec")
            if use_fast_recip:
                nc.vector.reciprocal_approx_fast(rec[:], den[:])
            else:
                nc.vector.reciprocal(rec[:], den[:])
            resb = epi_p.tile([128, 256], fp32, tag="resb")
            eng.tensor_tensor(out=resb[:], in0=T2_all[:, sl], in1=rec[:],
                              op=Alu.mult)
            nc.vector.scalar_tensor_tensor(
                resb[:], resb[:], 1e-7, xres_t[:, sl], op0=Alu.add, op1=Alu.add)
            pending[B] = resb

        def emit_batch_pe(B):
            """Transpose + h1 matmul + stats for a completed combine batch."""
            resb = pending.pop(B)
            trh = ps_t.tile([128, 1024], fp32, tag="ps_t")
            for q in range(4):
                nc.tensor.matmul(trh[0:64, 128 * q:128 * (q + 1)],
                                 resb[:, 64 * q:64 * (q + 1)], id32_t[:],
                                 is_transpose=True, skip_group_check=True)
            nc.vector.tensor_copy(outT[:, 512 * B:512 * (B + 1)],
                                  trh[0:64, 0:512])
            nc.tensor.matmul(trh[:, 512:1024], w1_t[:],
                             outT[:, 512 * B:512 * (B + 1)],
                             start=True, stop=True, skip_group_check=True)
            h1sl = h1s_p.tile([128, 512], fp32, tag="h1s")
            nc.scalar.copy(h1sl[:], trh[:, 512:1024])
            nc.sync.dma_start(h1_d[:, 512 * B:512 * (B + 1)], h1sl[:])
            nc.vector.tensor_reduce(s1p[:, B:B + 1], h1sl[:],
                                    axis=mybir.AxisListType.X, op=Alu.add)
            sq = sq_p.tile([128, 512], fp32, tag="sq")
            nc.vector.scalar_tensor_tensor(
                sq[:], h1sl[:], 0.0, h1sl[:], op0=Alu.add, op1=Alu.mult,
                accum_out=s2p[:, B:B + 1])

        # ---- edge phase (chunks of 2 node-tiles = 2048 edges)
        # The S/T reduction matmuls for chunk j are emitted during chunk j+1
        # ("deferred") so the PE never head-of-line blocks on the
        # relu->exp->Pm chain; likewise the per-batch PE epilogue is emitted
        # two chunks after its DVE combine.
        s1_pair = [None, None]
        chunk_pq = {}            # chunk -> (P_t, Pm_t)

        def emit_chunk_front(j):
            xae_t = xae_p.tile([128, 1536], bf16, tag="xae")
            nc.sync.dma_start(xae_t[:], xae_d[:, j * 1536:(j + 1) * 1536])

            t_ps = ps_t.tile([128, 1024], fp32, tag="ps_t")
            for i in range(2):
                nc.tensor.matmul(t_ps[:, 512 * i:512 * (i + 1)], id16_t[:],
                                 xae_t[:, 512 * i:512 * (i + 1)],
                                 start=True, stop=False)
                for g in range(2):
                    nc.tensor.matmul(
                        t_ps[:, 512 * i + 256 * g:512 * i + 256 * (g + 1)],
                        xae_t[:, 1024 + 256 * i + 128 * g:
                              1024 + 256 * i + 128 * (g + 1)],
                        wbd_t[:], start=False, stop=(g == 1))

            # r = relu(t): 7 of 8 chunks on scalar, 1 of 8 on vector
            r_t = r_p.tile([128, 1024], fp16, tag="rr")
            if j % 8 == 7:
                nc.vector.tensor_scalar_max(r_t[:], t_ps[:], 0.0)
            else:
                nc.scalar.activation(r_t[:], t_ps[:], Act.Relu)
            # P = exp(r) (scalar); Pm = r * P (vector)
            P_t = pp_p.tile([128, 1024], bf16, tag="pp")
            nc.scalar.activation(P_t[:], r_t[:], Act.Exp)
            Pm_t = pm_p.tile([128, 1024], bf16, tag="pm")
            nc.vector.tensor_tensor(out=Pm_t[:], in0=r_t[:], in1=P_t[:],
                                    op=Alu.mult)
            chunk_pq[j] = (P_t, Pm_t)

        def emit_chunk_back(j):
            P_t, Pm_t = chunk_pq.pop(j)
            for i in range(2):
                T = 2 * j + i
                c = T % 4
                if c == 0:
                    s1_pair[0] = ps_s.tile([128, 512], fp32, tag="ps_s",
                                           name=f"S1_{T}")
                    s1_pair[1] = ps_u.tile([128, 512], fp32, tag="ps_u",
                                           name=f"T1_{T}")
                S1_ps, T1_ps = s1_pair
                nc.tensor.matmul(S1_ps[32 * c:32 * c + 32, :], bd_t[:],
                                 P_t[:, 512 * i:512 * (i + 1)],
                                 start=True, stop=True, tile_position=(0, 32 * c),
                                 skip_group_check=True)
                nc.tensor.matmul(T1_ps[32 * c:32 * c + 32, :], bd_t[:],
                                 Pm_t[:, 512 * i:512 * (i + 1)],
                                 start=True, stop=True, tile_position=(0, 32 * c),
                                 skip_group_check=True)

                if c == 3:
                    G = T // 4
                    nc.vector.tensor_reduce(
                        S2_all[:, G * 64:(G + 1) * 64],
                        S1_ps[:].rearrange("p (b h) -> p h b", h=H),
                        axis=mybir.AxisListType.X, op=Alu.add)
                    nc.vector.tensor_reduce(
                        T2_all[:, G * 64:(G + 1) * 64],
                        T1_ps[:].rearrange("p (b h) -> p h b", h=H),
                        axis=mybir.AxisListType.X, op=Alu.add)
                    if G % 4 == 3:
                        emit_batch_dve(G // 4)

        for j in range(nchunk):
            if j >= 10 and (j - 10) % 8 == 0:
                emit_batch_pe((j - 10) // 8)
            emit_chunk_front(j)
            if j >= 1:
                emit_chunk_back(j - 1)
        emit_chunk_back(nchunk - 1)
        for B in sorted(pending):
            emit_batch_pe(B)

        # ---- finalize BN stats
        s1 = sb.tile([128, 1], fp32)
        nc.vector.tensor_reduce(s1[:], s1p[:], axis=mybir.AxisListType.X,
                                op=Alu.add)
        s2 = sb.tile([128, 1], fp32)
        nc.vector.tensor_reduce(s2[:], s2p[:], axis=mybir.AxisListType.X,
                                op=Alu.add)
        stats = sb.tile([128, 2], fp32)
        nc.vector.tensor_copy(stats[:, 0:1], s1[:])
        nc.vector.tensor_copy(stats[:, 1:2], s2[:])
        nc.scalar.dma_start(st_d[:], stats[:])

    _legalize_multiwaits(nc)
    return nc


# ------------------------------------------------------------ phase-2 kernel
def _build_phase2(ntiles=NT):
    fp32 = mybir.dt.float32
    bf16 = mybir.dt.bfloat16
    Act = mybir.ActivationFunctionType
    npc = ntiles * 32
    nslice = npc // 512
    nc = bass.Bass()

    h1_d = nc.declare_dram_parameter("h1", [128, npc], fp32, isOutput=False)
    # cw2: [128, 66] = [scale | shift | W2 (bf16-packed as fp32 bits? no:
    # scale/shift fp32 cols 0-1, then W2 bf16 staged separately)
    ss_d = nc.declare_dram_parameter("ss", [128, 2], fp32, isOutput=False)
    w2_d = nc.declare_dram_parameter("w2", [128, 64], bf16, isOutput=False)
    out_d = nc.declare_dram_parameter("out", [64, npc], fp32, isOutput=True)

    with tile.TileContext(nc) as tc, ExitStack() as ctx:
        const = ctx.enter_context(tc.tile_pool(name="const", bufs=1))
        h1_p = ctx.enter_context(tc.tile_pool(name="h1p", bufs=4))
        h2_p = ctx.enter_context(tc.tile_pool(name="h2p", bufs=3))
        osl_p = ctx.enter_context(tc.tile_pool(name="osl", bufs=3))
        ps = ctx.enter_context(tc.tile_pool(name="ps", bufs=3, space="PSUM"))

        ss_t = const.tile([128, 2], fp32)
        nc.sync.dma_start(ss_t[:], ss_d[:])
        w2_t = const.tile([128, 64], bf16)
        nc.sync.dma_start(w2_t[:], w2_d[:])

        pro_ps = ps.tile([128, 512], fp32, tag="ps")
        nc.tensor.matmul(pro_ps[0:8, 0:8], w2_t[:, 0:8], w2_t[:, 0:8],
                         start=True, stop=True, skip_group_check=True)

        for j in range(nslice):
            h1sl = h1_p.tile([128, 512], fp32, tag="h1p")
            nc.sync.dma_start(h1sl[:], h1_d[:, j * 512:(j + 1) * 512])
            h2sl = h2_p.tile([128, 512], bf16, tag="h2p")
            nc.scalar.activation(h2sl[:], h1sl[:], Act.Relu, bias=ss_t[:, 1:2],
                                 scale=ss_t[:, 0:1])
            o_ps = ps.tile([128, 512], fp32, tag="ps")
            nc.tensor.matmul(o_ps[0:64, :], w2_t[:], h2sl[:],
                             start=True, stop=True, skip_group_check=True)
            oslice = osl_p.tile([64, 512], fp32, tag="osl")
            nc.vector.tensor_copy(oslice[:], o_ps[0:64, :])
            nc.sync.dma_start(out_d[:, j * 512:(j + 1) * 512], oslice[:])

    _legalize_multiwaits(nc)
    return nc


# -------------------------------------------------------------- host staging
def _stage_core(x_c, xs_slot_c, ea_slot_c, ninv_c, ntiles=NT):
    """xs_slot_c: [npc, K, H] f32 (x[src], invalid slots = NEG_BIG)
    ea_slot_c: [npc, K, ED] f32;  ninv_c: [npc] f32.

    Returns (xae bf16 [128, nchunk*1536], corr f32, xres f32)."""
    a = xs_slot_c.reshape(ntiles, 32, 8, 4, H)          # [T, m, b, j, h]
    xs_dev = np.ascontiguousarray(
        a.transpose(1, 3, 0, 2, 4)).reshape(128, ntiles * 512)

    b = ea_slot_c.reshape(ntiles, 32, 8, 4, ED)         # [T, m, b, j, d]
    ea4 = np.ascontiguousarray(
        b.transpose(2, 4, 0, 1, 3)                      # [b, d, T, m, j]
        .reshape(2, 4, ED, ntiles, 128)                 # [g, r, d, T, e']
        .transpose(1, 2, 3, 0, 4)                       # [r, d, T, g, e']
    ).reshape(128, ntiles * 256)

    # fuse xs + ea into one per-chunk DMA block:
    # chunk j: [xs(2j) 512 | xs(2j+1) 512 | ea(2j) 256 | ea(2j+1) 256]
    nch = ntiles // 2
    xs3 = xs_dev.reshape(128, nch, 1024)
    ea3 = ea4.reshape(128, nch, 512)
    xae = np.concatenate([xs3, ea3], axis=2).astype(np_bf16)
    xae = np.ascontiguousarray(xae).reshape(128, nch * 1536)

    # node n = 128*G + p'  (p' = 32*(T%4) + node%32)
    corr = (ninv_c.astype(np.float32) - 1e-16)[:, None] * np.ones((1, H), np.float32)
    corr_dev = np.ascontiguousarray(
        corr.reshape(ntiles // 4, 128, H).transpose(1, 0, 2)).reshape(128, -1)
    xres_dev = np.ascontiguousarray(
        x_c.reshape(ntiles // 4, 128, H).transpose(1, 0, 2)).reshape(128, -1)
    return xae, corr_dev, xres_dev


def _consts(W_edge):
    Wbd = np.zeros((128, 256), np.float32)
    for r in range(4):
        Wbd[32 * r:32 * r + 32, 64 * r:64 * r + 64] = W_edge
    BD = np.zeros((128, 32), np.float32)
    for m in range(32):
        BD[4 * m:4 * m + 4, m] = 1.0
    ident = np.eye(128, dtype=np.float32)
    return Wbd, BD, ident


def build_in_maps(x, edge_index, edge_attr, nbr, W_edge, W1):
    """Stage the full inputs into per-core phase-1 input maps."""
    x = np.ascontiguousarray(np.asarray(x, dtype=np.float32))
    edge_attr = np.ascontiguousarray(np.asarray(edge_attr, dtype=np.float32))
    W_edge = np.ascontiguousarray(np.asarray(W_edge, dtype=np.float32))
    W1 = np.ascontiguousarray(np.asarray(W1, dtype=np.float32))

    src = np.asarray(edge_index[0], dtype=np.int64)
    nbr = np.asarray(nbr)
    valid = nbr >= 0                                    # [N, K]
    expect = np.arange(E, dtype=np.int64).reshape(N, K)
    assert np.array_equal(np.where(valid, nbr, expect), expect), \
        "kernel assumes nbr[i,k] == i*K+k on valid slots"

    src_slot = src.reshape(N, K)
    xs_slot = x[src_slot]                               # host halo: [N, K, H]
    xs_slot[~valid] = NEG_BIG
    ninv = (~valid).sum(axis=1).astype(np.float32)      # [N]
    ea_slot = edge_attr.reshape(N, K, ED)

    Wbd, BD, ident = _consts(W_edge)
    Wbd16 = Wbd.astype(np_bf16)
    BD16 = BD.astype(np_bf16)
    id16 = ident.astype(np_bf16)

    in_maps = []
    for core in range(NCORES):
        sl = slice(core * NPC, (core + 1) * NPC)
        xae, corr_dev, xres_dev = _stage_core(
            x[sl], xs_slot[sl], ea_slot[sl], ninv[sl])
        in_maps.append({
            "xae": xae, "wbd": Wbd16, "bd": BD16, "id16": id16,
            "id32": ident, "corr": corr_dev, "xres": xres_dev, "w1": W1,
        })
    return in_maps


def kernel(x, edge_index, edge_attr, nbr, W_edge, W1, gamma, beta, W2):
    gamma = np.asarray(gamma, dtype=np.float32)
    beta = np.asarray(beta, dtype=np.float32)
    W2 = np.ascontiguousarray(np.asarray(W2, dtype=np.float32))

    in_maps = build_in_maps(x, edge_index, edge_attr, nbr, W_edge, W1)

    if "p1" not in _compiled:
        _compiled["p1"] = _build_phase1(NT)
        _compiled["p2"] = _build_phase2(NT)

    res1 = run_bass_kernel_spmd(_compiled["p1"], in_maps,
                                core_ids=list(range(NCORES)))

    # host: combine BN stats (tiny 128-vector arithmetic), build scale/shift
    s1 = np.zeros(2 * H, np.float64)
    s2 = np.zeros(2 * H, np.float64)
    for core in range(NCORES):
        st = res1.results[core]["stats"].astype(np.float64)
        s1 += st[:, 0]
        s2 += st[:, 1]
    mean = (s1 / N).astype(np.float32)
    var = (s2 / N).astype(np.float32) - mean * mean
    scale = gamma / np.sqrt(var + 1e-5)
    shift = beta - mean * scale
    ss = np.stack([scale, shift], axis=1).astype(np.float32)  # [128, 2]

    W2_16 = W2.astype(np_bf16)
    in_maps2 = [{"h1": res1.results[core]["h1"], "ss": ss, "w2": W2_16}
                for core in range(NCORES)]
    res2 = run_bass_kernel_spmd(_compiled["p2"], in_maps2,
                                core_ids=list(range(NCORES)))

    out = np.empty((N, H), np.float32)
    for core in range(NCORES):
        out[core * NPC:(core + 1) * NPC] = res2.results[core]["out"].T
    return out


# revision 26
# speedup vs baseline: 1.3853x; 1.1427x over previous
"""Trainium2 Bass kernel for nn_ExportableGENConv (GENConv message passing +
channelwise softmax aggregation + MLP with global-batch BatchNorm), sharded
across 8 NeuronCores.

Contract: kernel(**inputs) takes the FULL inputs of reference.setup_inputs()
and returns the FULL [32768, 64] float32 output.

Sharding: nodes (each with K=32 contiguous incoming edge slots) are split
across 8 cores. Per-edge source features x[src] are materialized host-side
during staging (the halo exchange) in bf16, the per-edge message + per-node
softmax + MLP run on device. Global BatchNorm statistics are combined on host
between two NEFF launches: phase 1 produces h1 = (aggregated+x) @ W1 plus
per-core sum/sumsq, phase 2 applies the batch-norm affine + ReLU + W2.

Math (per node i, channel h, over valid slots k):
  t = x[src] + ea @ W_edge            (invalid slots: staged x = -1e9 -> t << 0)
  reference: m = relu(t) + 1e-7; softmax over k of m; res = sum m*alpha.
  With r = relu(t):  res = (sum_k r*e^r) / (sum_k e^r + 1e-16) + 1e-7
  Device: r = relu(t) (fp16); P = exp(r) (bf16; invalid slots contribute
  exactly 1.0, removed via a host-staged per-node count); Pm = r*P (bf16).

Device layout ("k-layout"): node-tile = 32 nodes = 1024 edge slots laid out as
partition p = (node%32)*4 + (k%4), free = (b=k//4 in [0,8), h). Tiles are
processed in chunks of 2 (one [128,1024] 2-bank PSUM tile per chunk) so the
scalar/vector elementwise passes amortize their fixed access latency. The
softmax reduction is a PE matmul with a block-diagonal ones stationary over
partition blocks of 4 (stage 1, bf16 PSUM out, col-offset-stacked over 4
node-tiles) + a DVE reduce over b (stage 2, innermost-strided AP). Engine
balance: relu alternates scalar(3):vector(1); Pm alternates vector/gpsimd;
the per-node combine + transpose + h1 matmul + BN-stats epilogue is pipelined
into the loop in batches of 4 node-groups (emitted one batch late so the PE
never head-of-line blocks on the DVE combine chain).
"""

import numpy as np
from contextlib import ExitStack

import concourse.bass as bass
import concourse.tile as tile
from concourse import mybir
from concourse.bass_utils import run_bass_kernel_spmd

try:
    from ml_dtypes import bfloat16 as np_bf16
except ImportError:  # ml_dtypes ships with jax; fall back just in case
    import jax.numpy as _jnp

    np_bf16 = _jnp.bfloat16

# ---------------------------------------------------------------- constants
N, K, H, ED = 32768, 32, 64, 32
E = N * K
NCORES = 8
NPC = N // NCORES            # nodes per core = 4096
NT = NPC // 32               # node-tiles per core = 128
NCH = NT // 2                # 2-tile chunks per core = 64
NEG_BIG = -1.0e9

_compiled = {}


# ------------------------------------------------------- multi-wait legalizer
def _legalize_multiwaits(nc):
    """This walrus build accepts only ONE sync wait per instruction; move the
    excess onto injected same-engine drain carriers placed immediately before
    the instruction (semantics-preserving: the engine stalls there instead)."""
    n_injected = 0
    for fn in nc.m.functions:
        for blk in fn.blocks:
            bb = blk if hasattr(blk, "instructions") else blk.bb
            insts = list(bb.instructions)
            out = []
            for inst in insts:
                si = inst.sync_info
                if si is not None and si.on_wait and len(si.on_wait) > 1:
                    waits = list(si.on_wait)
                    for w in waits[:-1]:
                        nop = mybir.InstDrain(
                            name=f"waitfix-{nc.next_id()}", ins=[], outs=[]
                        )
                        nop.engine = inst.engine
                        nop.sync_info = mybir.SyncInfo(on_wait=[w], on_update=[])
                        nc.register_instruction(nop, overwrite=True)
                        out.append(nop)
                        n_injected += 1
                    inst.sync_info = mybir.SyncInfo(
                        on_wait=[waits[-1]], on_update=list(si.on_update or [])
                    )
                out.append(inst)
            if len(out) != len(insts):
                bb.instructions = out
    return n_injected


# ------------------------------------------------------------ phase-1 kernel
def _build_phase1(ntiles=NT, use_gpsimd=False, use_fast_recip=False):
    fp32 = mybir.dt.float32
    bf16 = mybir.dt.bfloat16
    fp16 = mybir.dt.float16
    Act = mybir.ActivationFunctionType
    Alu = mybir.AluOpType
    npc = ntiles * 32
    ngrp = ntiles // 4
    nchunk = ntiles // 2
    nbatch = ngrp // 4           # epilogue batches (4 node-groups each)
    nc = bass.Bass()

    xae_d = nc.declare_dram_parameter("xae", [128, nchunk * 1536], bf16,
                                      isOutput=False)
    wbd_d = nc.declare_dram_parameter("wbd", [128, 256], bf16, isOutput=False)
    bd_d = nc.declare_dram_parameter("bd", [128, 32], bf16, isOutput=False)
    id16_d = nc.declare_dram_parameter("id16", [128, 128], bf16, isOutput=False)
    id32_d = nc.declare_dram_parameter("id32", [128, 128], fp32, isOutput=False)
    corr_d = nc.declare_dram_parameter("corr", [128, ngrp * 64], fp32, isOutput=False)
    xres_d = nc.declare_dram_parameter("xres", [128, ngrp * 64], fp32, isOutput=False)
    w1_d = nc.declare_dram_parameter("w1", [64, 128], fp32, isOutput=False)
    h1_d = nc.declare_dram_parameter("h1", [128, npc], fp32, isOutput=True)
    st_d = nc.declare_dram_parameter("stats", [128, 2], fp32, isOutput=True)

    with tile.TileContext(nc) as tc, ExitStack() as ctx:
        const = ctx.enter_context(tc.tile_pool(name="const", bufs=1))
        sb = ctx.enter_context(tc.tile_pool(name="sb", bufs=1))
        xae_p = ctx.enter_context(tc.tile_pool(name="xae", bufs=4))
        r_p = ctx.enter_context(tc.tile_pool(name="rr", bufs=4))
        pp_p = ctx.enter_context(tc.tile_pool(name="pp", bufs=4))
        pm_p = ctx.enter_context(tc.tile_pool(name="pm", bufs=4))
        epi_p = ctx.enter_context(tc.tile_pool(name="epi", bufs=2))
        h1s_p = ctx.enter_context(tc.tile_pool(name="h1s", bufs=2))
        sq_p = ctx.enter_context(tc.tile_pool(name="sq", bufs=2))
        ps_t = ctx.enter_context(tc.tile_pool(name="ps_t", bufs=2, space="PSUM"))
        ps_s = ctx.enter_context(tc.tile_pool(name="ps_s", bufs=1, space="PSUM"))
        ps_u = ctx.enter_context(tc.tile_pool(name="ps_u", bufs=1, space="PSUM"))
        ps_e = ctx.enter_context(tc.tile_pool(name="ps_e", bufs=1, space="PSUM"))

        wbd_t = const.tile([128, 256], bf16)
        nc.sync.dma_start(wbd_t[:], wbd_d[:])
        bd_t = const.tile([128, 32], bf16)
        nc.sync.dma_start(bd_t[:], bd_d[:])
        id16_t = const.tile([128, 128], bf16)
        nc.sync.dma_start(id16_t[:], id16_d[:])
        id32_t = const.tile([128, 128], fp32)
        nc.sync.dma_start(id32_t[:], id32_d[:])
        corr_t = const.tile([128, ngrp * 64], fp32)
        nc.sync.dma_start(corr_t[:], corr_d[:])
        xres_t = const.tile([128, ngrp * 64], fp32)
        nc.sync.dma_start(xres_t[:], xres_d[:])
        w1_t = const.tile([64, 128], fp32)
        nc.sync.dma_start(w1_t[:], w1_d[:])

        S2_all = sb.tile([128, ngrp * 64], fp32)
        T2_all = sb.tile([128, ngrp * 64], fp32)
        outT = sb.tile([64, npc], fp32)
        s1p = sb.tile([128, nbatch], fp32)
        s2p = sb.tile([128, nbatch], fp32)

        # prologue: make PE observe const DMA sems via tiny touch matmuls
        pro_ps = ps_s.tile([128, 512], fp32, tag="ps_s")
        nc.tensor.matmul(pro_ps[0:8, 0:8], id16_t[:, 0:8], id16_t[:, 0:8],
                         start=True, stop=True, skip_group_check=True)
        nc.tensor.matmul(pro_ps[0:8, 0:8], wbd_t[:, 0:8], wbd_t[:, 0:8],
                         start=True, stop=True, skip_group_check=True)
        nc.tensor.matmul(pro_ps[0:8, 0:8], bd_t[:, 0:8], bd_t[:, 0:8],
                         start=True, stop=True, skip_group_check=True)
        nc.tensor.matmul(pro_ps[0:8, 0:8], id32_t[:, 0:8], id32_t[:, 0:8],
                         start=True, stop=True, skip_group_check=True)
        nc.tensor.matmul(pro_ps[0:8, 0:8], w1_t[:, 0:8], w1_t[:, 0:8],
                         start=True, stop=True, skip_group_check=True)

        pending = {}             # batch id -> res tile awaiting PE epilogue

        def emit_batch_dve(B):
            """Per-node combine for node-groups 4B..4B+3 (gpsimd + DVE recip)."""
            sl = slice(B * 256, (B + 1) * 256)
            eng = nc.gpsimd if use_gpsimd else nc.vector
            den = epi_p.tile([128, 256], fp32, tag="den")
            eng.tensor_tensor(out=den[:], in0=S2_all[:, sl],
                              in1=corr_t[:, sl], op=Alu.subtract)
            rec = epi_p.tile([128, 256], fp32, tag="rec")
            if use_fast_recip:
                nc.vector.reciprocal_approx_fast(rec[:], den[:])
            else:
                nc.vector.reciprocal(rec[:], den[:])
            resb = epi_p.tile([128, 256], fp32, tag="resb")
            eng.tensor_tensor(out=resb[:], in0=T2_all[:, sl], in1=rec[:],
                              op=Alu.mult)
            nc.vector.scalar_tensor_tensor(
                resb[:], resb[:], 1e-7, xres_t[:, sl], op0=Alu.add, op1=Alu.add)
            pending[B] = resb

        def emit_batch_pe(B):
            """Transpose + h1 matmul + stats for a completed combine batch."""
            resb = pending.pop(B)
            trh = ps_e.tile([128, 1024], fp32, tag="ps_e")
            for q in range(4):
                nc.tensor.matmul(trh[0:64, 128 * q:128 * (q + 1)],
                                 resb[:, 64 * q:64 * (q + 1)], id32_t[:],
                                 is_transpose=True, skip_group_check=True)
            nc.vector.tensor_copy(outT[:, 512 * B:512 * (B + 1)],
                                  trh[0:64, 0:512])
            nc.tensor.matmul(trh[:, 512:1024], w1_t[:],
                             outT[:, 512 * B:512 * (B + 1)],
                             start=True, stop=True, skip_group_check=True)
            h1sl = h1s_p.tile([128, 512], fp32, tag="h1s")
            nc.scalar.copy(h1sl[:], trh[:, 512:1024])
            nc.sync.dma_start(h1_d[:, 512 * B:512 * (B + 1)], h1sl[:])
            nc.vector.tensor_reduce(s1p[:, B:B + 1], h1sl[:],
                                    axis=mybir.AxisListType.X, op=Alu.add)
            sq = sq_p.tile([128, 512], fp32, tag="sq")
            nc.vector.scalar_tensor_tensor(
                sq[:], h1sl[:], 0.0, h1sl[:], op0=Alu.add, op1=Alu.mult,
                accum_out=s2p[:, B:B + 1])

        # ---- edge phase (chunks of 2 node-tiles = 2048 edges)
        # The S/T reduction matmuls for chunk j are emitted during chunk j+1
        # ("deferred") so the PE never head-of-line blocks on the
        # relu->exp->Pm chain; likewise the per-batch PE epilogue is emitted
        # two chunks after its DVE combine.
        s1_pair = [None, None]
        chunk_pq = {}            # chunk -> (P_t, Pm_t)

        xae_tiles = {}

        def issue_dma(j):
            if j >= nchunk:
                return
            xae_t = xae_p.tile([128, 1536], bf16, tag="xae")
            nc.sync.dma_start(xae_t[:], xae_d[:, j * 1536:(j + 1) * 1536])
            xae_tiles[j] = xae_t

        def emit_chunk_front(j):
            xae_t = xae_tiles.pop(j)
            t_ps = ps_t.tile([128, 1024], fp32, tag="ps_t")
            # one id16 LDWEIGHTS for both tiles, then the 4 ea stationaries
            for i in range(2):
                nc.tensor.matmul(t_ps[:, 512 * i:512 * (i + 1)], id16_t[:],
                                 xae_t[:, 512 * i:512 * (i + 1)],
                                 start=True, stop=False, skip_group_check=True)
            for i in range(2):
                for g in range(2):
                    nc.tensor.matmul(
                        t_ps[:, 512 * i + 256 * g:512 * i + 256 * (g + 1)],
                        xae_t[:, 1024 + 256 * i + 128 * g:
                              1024 + 256 * i + 128 * (g + 1)],
                        wbd_t[:], start=False, stop=(g == 1),
                        skip_group_check=True)

            # r = relu(t): 7 of 8 chunks on scalar, 1 of 8 on vector
            r_t = r_p.tile([128, 1024], fp16, tag="rr")
            if j % 8 == 7:
                nc.vector.tensor_scalar_max(r_t[:], t_ps[:], 0.0)
            else:
                nc.scalar.activation(r_t[:], t_ps[:], Act.Relu)
            # P = exp(r) (scalar); Pm = r * P (vector)
            P_t = pp_p.tile([128, 1024], bf16, tag="pp")
            nc.scalar.activation(P_t[:], r_t[:], Act.Exp)
            Pm_t = pm_p.tile([128, 1024], bf16, tag="pm")
            nc.vector.tensor_tensor(out=Pm_t[:], in0=r_t[:], in1=P_t[:],
                                    op=Alu.mult)
            chunk_pq[j] = (P_t, Pm_t)

        def emit_chunk_back(j):
            P_t, Pm_t = chunk_pq.pop(j)
            for i in range(2):
                T = 2 * j + i
                c = T % 4
                if c == 0:
                    s1_pair[0] = ps_s.tile([128, 512], fp32, tag="ps_s",
                                           name=f"S1_{T}")
                    s1_pair[1] = ps_u.tile([128, 512], fp32, tag="ps_u",
                                           name=f"T1_{T}")
                S1_ps, T1_ps = s1_pair
                nc.tensor.matmul(S1_ps[32 * c:32 * c + 32, :], bd_t[:],
                                 P_t[:, 512 * i:512 * (i + 1)],
                                 start=True, stop=True, tile_position=(0, 32 * c),
                                 skip_group_check=True)
                nc.tensor.matmul(T1_ps[32 * c:32 * c + 32, :], bd_t[:],
                                 Pm_t[:, 512 * i:512 * (i + 1)],
                                 start=True, stop=True, tile_position=(0, 32 * c),
                                 skip_group_check=True)

                if c == 3:
                    G = T // 4
                    nc.vector.tensor_reduce(
                        S2_all[:, G * 64:(G + 1) * 64],
                        S1_ps[:].rearrange("p (b h) -> p h b", h=H),
                        axis=mybir.AxisListType.X, op=Alu.add)
                    nc.vector.tensor_reduce(
                        T2_all[:, G * 64:(G + 1) * 64],
                        T1_ps[:].rearrange("p (b h) -> p h b", h=H),
                        axis=mybir.AxisListType.X, op=Alu.add)
                    if G % 4 == 3:
                        emit_batch_dve(G // 4)

        issue_dma(0)
        issue_dma(1)
        for j in range(nchunk):
            if j >= 10 and (j - 10) % 8 == 0:
                emit_batch_pe((j - 10) // 8)
            issue_dma(j + 2)
            emit_chunk_front(j)
            if j >= 1:
                emit_chunk_back(j - 1)
        emit_chunk_back(nchunk - 1)
        for B in sorted(pending):
            emit_batch_pe(B)

        # ---- finalize BN stats
        s1 = sb.tile([128, 1], fp32)
        nc.vector.tensor_reduce(s1[:], s1p[:], axis=mybir.AxisListType.X,
                                op=Alu.add)
        s2 = sb.tile([128, 1], fp32)
        nc.vector.tensor_reduce(s2[:], s2p[:], axis=mybir.AxisListType.X,
                                op=Alu.add)
        stats = sb.tile([128, 2], fp32)
        nc.vector.tensor_copy(stats[:, 0:1], s1[:])
        nc.vector.tensor_copy(stats[:, 1:2], s2[:])
        nc.scalar.dma_start(st_d[:], stats[:])

    _legalize_multiwaits(nc)
    return nc


# ------------------------------------------------------------ phase-2 kernel
def _build_phase2(ntiles=NT):
    fp32 = mybir.dt.float32
    bf16 = mybir.dt.bfloat16
    Act = mybir.ActivationFunctionType
    npc = ntiles * 32
    nslice = npc // 512
    nc = bass.Bass()

    h1_d = nc.declare_dram_parameter("h1", [128, npc], fp32, isOutput=False)
    # cw2: [128, 66] = [scale | shift | W2 (bf16-packed as fp32 bits? no:
    # scale/shift fp32 cols 0-1, then W2 bf16 staged separately)
    ss_d = nc.declare_dram_parameter("ss", [128, 2], fp32, isOutput=False)
    w2_d = nc.declare_dram_parameter("w2", [128, 64], bf16, isOutput=False)
    out_d = nc.declare_dram_parameter("out", [64, npc], fp32, isOutput=True)

    with tile.TileContext(nc) as tc, ExitStack() as ctx:
        const = ctx.enter_context(tc.tile_pool(name="const", bufs=1))
        h1_p = ctx.enter_context(tc.tile_pool(name="h1p", bufs=4))
        h2_p = ctx.enter_context(tc.tile_pool(name="h2p", bufs=3))
        osl_p = ctx.enter_context(tc.tile_pool(name="osl", bufs=3))
        ps = ctx.enter_context(tc.tile_pool(name="ps", bufs=3, space="PSUM"))

        ss_t = const.tile([128, 2], fp32)
        nc.sync.dma_start(ss_t[:], ss_d[:])
        w2_t = const.tile([128, 64], bf16)
        nc.sync.dma_start(w2_t[:], w2_d[:])

        pro_ps = ps.tile([128, 512], fp32, tag="ps")
        nc.tensor.matmul(pro_ps[0:8, 0:8], w2_t[:, 0:8], w2_t[:, 0:8],
                         start=True, stop=True, skip_group_check=True)

        for j in range(nslice):
            h1sl = h1_p.tile([128, 512], fp32, tag="h1p")
            nc.sync.dma_start(h1sl[:], h1_d[:, j * 512:(j + 1) * 512])
            h2sl = h2_p.tile([128, 512], bf16, tag="h2p")
            nc.scalar.activation(h2sl[:], h1sl[:], Act.Relu, bias=ss_t[:, 1:2],
                                 scale=ss_t[:, 0:1])
            o_ps = ps.tile([128, 512], fp32, tag="ps")
            nc.tensor.matmul(o_ps[0:64, :], w2_t[:], h2sl[:],
                             start=True, stop=True, skip_group_check=True)
            oslice = osl_p.tile([64, 512], fp32, tag="osl")
            nc.vector.tensor_copy(oslice[:], o_ps[0:64, :])
            nc.sync.dma_start(out_d[:, j * 512:(j + 1) * 512], oslice[:])

    _legalize_multiwaits(nc)
    return nc


# -------------------------------------------------------------- host staging
def _stage_core(x_c, xs_slot_c, ea_slot_c, ninv_c, ntiles=NT):
    """xs_slot_c: [npc, K, H] f32 (x[src], invalid slots = NEG_BIG)
    ea_slot_c: [npc, K, ED] f32;  ninv_c: [npc] f32.

    Returns (xae bf16 [128, nchunk*1536], corr f32, xres f32)."""
    a = xs_slot_c.reshape(ntiles, 32, 8, 4, H)          # [T, m, b, j, h]
    xs_dev = np.ascontiguousarray(
        a.transpose(1, 3, 0, 2, 4)).reshape(128, ntiles * 512)

    b = ea_slot_c.reshape(ntiles, 32, 8, 4, ED)         # [T, m, b, j, d]
    ea4 = np.ascontiguousarray(
        b.transpose(2, 4, 0, 1, 3)                      # [b, d, T, m, j]
        .reshape(2, 4, ED, ntiles, 128)                 # [g, r, d, T, e']
        .transpose(1, 2, 3, 0, 4)                       # [r, d, T, g, e']
    ).reshape(128, ntiles * 256)

    # fuse xs + ea into one per-chunk DMA block:
    # chunk j: [xs(2j) 512 | xs(2j+1) 512 | ea(2j) 256 | ea(2j+1) 256]
    nch = ntiles // 2
    xs3 = xs_dev.reshape(128, nch, 1024)
    ea3 = ea4.reshape(128, nch, 512)
    xae = np.concatenate([xs3, ea3], axis=2).astype(np_bf16)
    xae = np.ascontiguousarray(xae).reshape(128, nch * 1536)

    # node n = 128*G + p'  (p' = 32*(T%4) + node%32)
    corr = (ninv_c.astype(np.float32) - 1e-16)[:, None] * np.ones((1, H), np.float32)
    corr_dev = np.ascontiguousarray(
        corr.reshape(ntiles // 4, 128, H).transpose(1, 0, 2)).reshape(128, -1)
    xres_dev = np.ascontiguousarray(
        x_c.reshape(ntiles // 4, 128, H).transpose(1, 0, 2)).reshape(128, -1)
    return xae, corr_dev, xres_dev


def _consts(W_edge):
    Wbd = np.zeros((128, 256), np.float32)
    for r in range(4):
        Wbd[32 * r:32 * r + 32, 64 * r:64 * r + 64] = W_edge
    BD = np.zeros((128, 32), np.float32)
    for m in range(32):
        BD[4 * m:4 * m + 4, m] = 1.0
    ident = np.eye(128, dtype=np.float32)
    return Wbd, BD, ident


def build_in_maps(x, edge_index, edge_attr, nbr, W_edge, W1):
    """Stage the full inputs into per-core phase-1 input maps."""
    x = np.ascontiguousarray(np.asarray(x, dtype=np.float32))
    edge_attr = np.ascontiguousarray(np.asarray(edge_attr, dtype=np.float32))
    W_edge = np.ascontiguousarray(np.asarray(W_edge, dtype=np.float32))
    W1 = np.ascontiguousarray(np.asarray(W1, dtype=np.float32))

    src = np.asarray(edge_index[0], dtype=np.int64)
    nbr = np.asarray(nbr)
    valid = nbr >= 0                                    # [N, K]
    expect = np.arange(E, dtype=np.int64).reshape(N, K)
    assert np.array_equal(np.where(valid, nbr, expect), expect), \
        "kernel assumes nbr[i,k] == i*K+k on valid slots"

    src_slot = src.reshape(N, K)
    xs_slot = x[src_slot]                               # host halo: [N, K, H]
    xs_slot[~valid] = NEG_BIG
    ninv = (~valid).sum(axis=1).astype(np.float32)      # [N]
    ea_slot = edge_attr.reshape(N, K, ED)

    Wbd, BD, ident = _consts(W_edge)
    Wbd16 = Wbd.astype(np_bf16)
    BD16 = BD.astype(np_bf16)
    id16 = ident.astype(np_bf16)

    in_maps = []
    for core in range(NCORES):
        sl = slice(core * NPC, (core + 1) * NPC)
        xae, corr_dev, xres_dev = _stage_core(
            x[sl], xs_slot[sl], ea_slot[sl], ninv[sl])
        in_maps.append({
            "xae": xae, "wbd": Wbd16, "bd": BD16, "id16": id16,
            "id32": ident, "corr": corr_dev, "xres": xres_dev, "w1": W1,
        })
    return in_maps


def kernel(x, edge_index, edge_attr, nbr, W_edge, W1, gamma, beta, W2):
    gamma = np.asarray(gamma, dtype=np.float32)
    beta = np.asarray(beta, dtype=np.float32)
    W2 = np.ascontiguousarray(np.asarray(W2, dtype=np.float32))

    in_maps = build_in_maps(x, edge_index, edge_attr, nbr, W_edge, W1)

    if "p1" not in _compiled:
        _compiled["p1"] = _build_phase1(NT)
        _compiled["p2"] = _build_phase2(NT)

    res1 = run_bass_kernel_spmd(_compiled["p1"], in_maps,
                                core_ids=list(range(NCORES)))

    # host: combine BN stats (tiny 128-vector arithmetic), build scale/shift
    s1 = np.zeros(2 * H, np.float64)
    s2 = np.zeros(2 * H, np.float64)
    for core in range(NCORES):
        st = res1.results[core]["stats"].astype(np.float64)
        s1 += st[:, 0]
        s2 += st[:, 1]
    mean = (s1 / N).astype(np.float32)
    var = (s2 / N).astype(np.float32) - mean * mean
    scale = gamma / np.sqrt(var + 1e-5)
    shift = beta - mean * scale
    ss = np.stack([scale, shift], axis=1).astype(np.float32)  # [128, 2]

    W2_16 = W2.astype(np_bf16)
    in_maps2 = [{"h1": res1.results[core]["h1"], "ss": ss, "w2": W2_16}
                for core in range(NCORES)]
    res2 = run_bass_kernel_spmd(_compiled["p2"], in_maps2,
                                core_ids=list(range(NCORES)))

    out = np.empty((N, H), np.float32)
    for core in range(NCORES):
        out[core * NPC:(core + 1) * NPC] = res2.results[core]["out"].T
    return out


# revision 32
# speedup vs baseline: 1.4679x; 1.0597x over previous
"""Trainium2 Bass kernel for nn_ExportableGENConv (GENConv message passing +
channelwise softmax aggregation + MLP with global-batch BatchNorm), sharded
across 8 NeuronCores.

Contract: kernel(**inputs) takes the FULL inputs of reference.setup_inputs()
and returns the FULL [32768, 64] float32 output.

Sharding: nodes (each with K=32 contiguous incoming edge slots) are split
across 8 cores. Per-edge source features x[src] are materialized host-side
during staging (the halo exchange) in bf16, the per-edge message + per-node
softmax + MLP run on device. Global BatchNorm statistics are combined on host
between two NEFF launches: phase 1 produces h1 = (aggregated+x) @ W1 plus
per-core sum/sumsq, phase 2 applies the batch-norm affine + ReLU + W2.

Math (per node i, channel h, over valid slots k):
  t = x[src] + ea @ W_edge            (invalid slots: staged x = -1e9 -> t << 0)
  reference: m = relu(t) + 1e-7; softmax over k of m; res = sum m*alpha.
  With r = relu(t):  res = (sum_k r*e^r) / (sum_k e^r + 1e-16) + 1e-7
  Device: r = relu(t) (fp16); P = exp(r) (bf16; invalid slots contribute
  exactly 1.0, removed via a host-staged per-node count); Pm = r*P (bf16).

Device layout ("k-layout"): node-tile = 32 nodes = 1024 edge slots laid out as
partition p = (node%32)*4 + (k%4), free = (b=k//4 in [0,8), h). Tiles are
processed in chunks of 2 (one [128,1024] 2-bank PSUM tile per chunk) so the
scalar/vector elementwise passes amortize their fixed access latency. The
softmax reduction is a PE matmul with a block-diagonal ones stationary over
partition blocks of 4 (stage 1, bf16 PSUM out, col-offset-stacked over 4
node-tiles) + a DVE reduce over b (stage 2, innermost-strided AP). Engine
balance: relu alternates scalar(3):vector(1); Pm alternates vector/gpsimd;
the per-node combine + transpose + h1 matmul + BN-stats epilogue is pipelined
into the loop in batches of 4 node-groups (emitted one batch late so the PE
never head-of-line blocks on the DVE combine chain).
"""

import numpy as np
from contextlib import ExitStack

import concourse.bass as bass
import concourse.tile as tile
from concourse import mybir
from concourse.bass_utils import run_bass_kernel_spmd

try:
    from ml_dtypes import bfloat16 as np_bf16
except ImportError:  # ml_dtypes ships with jax; fall back just in case
    import jax.numpy as _jnp

    np_bf16 = _jnp.bfloat16

# ---------------------------------------------------------------- constants
N, K, H, ED = 32768, 32, 64, 32
E = N * K
NCORES = 8
NPC = N // NCORES            # nodes per core = 4096
NT = NPC // 32               # node-tiles per core = 128
NCH = NT // 2                # 2-tile chunks per core = 64
NEG_BIG = -1.0e9

_compiled = {}


# ------------------------------------------------------- multi-wait legalizer
def _legalize_multiwaits(nc):
    """This walrus build accepts only ONE sync wait per instruction; move the
    excess onto injected same-engine drain carriers placed immediately before
    the instruction (semantics-preserving: the engine stalls there instead)."""
    n_injected = 0
    for fn in nc.m.functions:
        for blk in fn.blocks:
            bb = blk if hasattr(blk, "instructions") else blk.bb
            insts = list(bb.instructions)
            out = []
            for inst in insts:
                si = inst.sync_info
                if si is not None and si.on_wait and len(si.on_wait) > 1:
                    waits = list(si.on_wait)
                    for w in waits[:-1]:
                        nop = mybir.InstDrain(
                            name=f"waitfix-{nc.next_id()}", ins=[], outs=[]
                        )
                        nop.engine = inst.engine
                        nop.sync_info = mybir.SyncInfo(on_wait=[w], on_update=[])
                        nc.register_instruction(nop, overwrite=True)
                        out.append(nop)
                        n_injected += 1
                    inst.sync_info = mybir.SyncInfo(
                        on_wait=[waits[-1]], on_update=list(si.on_update or [])
                    )
                out.append(inst)
            if len(out) != len(insts):
                bb.instructions = out
    return n_injected


# ------------------------------------------------------------ phase-1 kernel
def _build_phase1(ntiles=NT, use_gpsimd=False, use_fast_recip=False):
    fp32 = mybir.dt.float32
    bf16 = mybir.dt.bfloat16
    fp16 = mybir.dt.float16
    Act = mybir.ActivationFunctionType
    Alu = mybir.AluOpType
    npc = ntiles * 32
    ngrp = ntiles // 4
    nchunk = ntiles // 2
    nbatch = ngrp // 4           # epilogue batches (4 node-groups each)
    nc = bass.Bass()

    xae_d = nc.declare_dram_parameter("xae", [128, nchunk * 1536], bf16,
                                      isOutput=False)
    wbd_d = nc.declare_dram_parameter("wbd", [128, 256], bf16, isOutput=False)
    bd_d = nc.declare_dram_parameter("bd", [128, 32], bf16, isOutput=False)
    id16_d = nc.declare_dram_parameter("id16", [128, 128], bf16, isOutput=False)
    id32_d = nc.declare_dram_parameter("id32", [128, 128], fp32, isOutput=False)
    corr_d = nc.declare_dram_parameter("corr", [128, ngrp * 64], fp32, isOutput=False)
    xres_d = nc.declare_dram_parameter("xres", [128, ngrp * 64], fp32, isOutput=False)
    w1_d = nc.declare_dram_parameter("w1", [64, 128], bf16, isOutput=False)
    h1_d = nc.declare_dram_parameter("h1", [128, npc], fp32, isOutput=True)
    st_d = nc.declare_dram_parameter("stats", [128, 2], fp32, isOutput=True)

    with tile.TileContext(nc) as tc, ExitStack() as ctx:
        const = ctx.enter_context(tc.tile_pool(name="const", bufs=1))
        sb = ctx.enter_context(tc.tile_pool(name="sb", bufs=1))
        xae_p = ctx.enter_context(tc.tile_pool(name="xae", bufs=4))
        r_p = ctx.enter_context(tc.tile_pool(name="rr", bufs=4))
        pp_p = ctx.enter_context(tc.tile_pool(name="pp", bufs=3))
        epi_p = ctx.enter_context(tc.tile_pool(name="epi", bufs=2))
        h1s_p = ctx.enter_context(tc.tile_pool(name="h1s", bufs=2))
        sq_p = ctx.enter_context(tc.tile_pool(name="sq", bufs=2))
        ps_t = ctx.enter_context(tc.tile_pool(name="ps_t", bufs=2, space="PSUM"))
        ps_s = ctx.enter_context(tc.tile_pool(name="ps_s", bufs=1, space="PSUM"))
        ps_u = ctx.enter_context(tc.tile_pool(name="ps_u", bufs=1, space="PSUM"))
        ps_e = ctx.enter_context(tc.tile_pool(name="ps_e", bufs=1, space="PSUM"))

        wbd_t = const.tile([128, 256], bf16)
        nc.sync.dma_start(wbd_t[:], wbd_d[:])
        bd_t = const.tile([128, 32], bf16)
        nc.sync.dma_start(bd_t[:], bd_d[:])
        id16_t = const.tile([128, 128], bf16)
        nc.sync.dma_start(id16_t[:], id16_d[:])
        id32_t = const.tile([128, 128], fp32)
        nc.sync.dma_start(id32_t[:], id32_d[:])
        corr_t = const.tile([128, ngrp * 64], fp32)
        nc.sync.dma_start(corr_t[:], corr_d[:])
        xres_t = const.tile([128, ngrp * 64], fp32)
        nc.sync.dma_start(xres_t[:], xres_d[:])
        w1_t = const.tile([64, 128], bf16)
        nc.sync.dma_start(w1_t[:], w1_d[:])

        S2_all = sb.tile([128, ngrp * 64], fp32)
        T2_all = sb.tile([128, ngrp * 64], fp32)
        outT = sb.tile([64, npc], bf16)
        s1p = sb.tile([128, nbatch], fp32)
        s2p = sb.tile([128, nbatch], fp32)

        # prologue: make PE observe const DMA sems via tiny touch matmuls
        pro_ps = ps_s.tile([128, 512], fp32, tag="ps_s")
        nc.tensor.matmul(pro_ps[0:8, 0:8], id16_t[:, 0:8], id16_t[:, 0:8],
                         start=True, stop=True, skip_group_check=True)
        nc.tensor.matmul(pro_ps[0:8, 0:8], wbd_t[:, 0:8], wbd_t[:, 0:8],
                         start=True, stop=True, skip_group_check=True)
        nc.tensor.matmul(pro_ps[0:8, 0:8], bd_t[:, 0:8], bd_t[:, 0:8],
                         start=True, stop=True, skip_group_check=True)
        nc.tensor.matmul(pro_ps[0:8, 0:8], id32_t[:, 0:8], id32_t[:, 0:8],
                         start=True, stop=True, skip_group_check=True)
        nc.tensor.matmul(pro_ps[0:8, 0:8], w1_t[:, 0:8], w1_t[:, 0:8],
                         start=True, stop=True, skip_group_check=True)

        pending = {}             # batch id -> res tile awaiting PE epilogue

        def emit_batch_dve(B):
            """Per-node combine for node-groups 4B..4B+3 (gpsimd + DVE recip)."""
            sl = slice(B * 256, (B + 1) * 256)
            eng = nc.gpsimd if use_gpsimd else nc.vector
            den = epi_p.tile([128, 256], fp32, tag="den")
            eng.tensor_tensor(out=den[:], in0=S2_all[:, sl],
                              in1=corr_t[:, sl], op=Alu.subtract)
            rec = epi_p.tile([128, 256], fp32, tag="rec")
            if use_fast_recip:
                nc.vector.reciprocal_approx_fast(rec[:], den[:])
            else:
                nc.vector.reciprocal(rec[:], den[:])
            resb = epi_p.tile([128, 256], fp32, tag="resb")
            eng.tensor_tensor(out=resb[:], in0=T2_all[:, sl], in1=rec[:],
                              op=Alu.mult)
            nc.vector.scalar_tensor_tensor(
                resb[:], resb[:], 1e-7, xres_t[:, sl], op0=Alu.add, op1=Alu.add)
            pending[B] = resb

        def emit_batch_pe(B):
            """Transpose + h1 matmul + stats for a completed combine batch."""
            resb = pending.pop(B)
            trh = ps_e.tile([128, 1024], fp32, tag="ps_e")
            for q in range(4):
                nc.tensor.matmul(trh[0:64, 128 * q:128 * (q + 1)],
                                 resb[:, 64 * q:64 * (q + 1)], id32_t[:],
                                 is_transpose=True, skip_group_check=True)
            nc.vector.tensor_copy(outT[:, 512 * B:512 * (B + 1)],
                                  trh[0:64, 0:512])
            nc.tensor.matmul(trh[:, 512:1024], w1_t[:],
                             outT[:, 512 * B:512 * (B + 1)],
                             start=True, stop=True, skip_group_check=True)
            h1sl = h1s_p.tile([128, 512], fp32, tag="h1s")
            nc.scalar.copy(h1sl[:], trh[:, 512:1024])
            nc.sync.dma_start(h1_d[:, 512 * B:512 * (B + 1)], h1sl[:])
            nc.vector.tensor_reduce(s1p[:, B:B + 1], h1sl[:],
                                    axis=mybir.AxisListType.X, op=Alu.add)
            sq = sq_p.tile([128, 512], fp32, tag="sq")
            nc.vector.scalar_tensor_tensor(
                sq[:], h1sl[:], 0.0, h1sl[:], op0=Alu.add, op1=Alu.mult,
                accum_out=s2p[:, B:B + 1])

        # ---- edge phase (chunks of 2 node-tiles = 2048 edges)
        # The S/T reduction matmuls for chunk j are emitted during chunk j+1
        # ("deferred") so the PE never head-of-line blocks on the
        # relu->exp->Pm chain; likewise the per-batch PE epilogue is emitted
        # two chunks after its DVE combine.
        s1_pair = [None, None]
        chunk_pq = {}            # chunk -> (P_t, Pm_t)

        xae_tiles = {}

        def issue_dma(j):
            if j >= nchunk:
                return
            xae_t = xae_p.tile([128, 1536], bf16, tag="xae")
            nc.sync.dma_start(xae_t[:], xae_d[:, j * 1536:(j + 1) * 1536])
            xae_tiles[j] = xae_t

        def emit_chunk_front(j):
            xae_t = xae_tiles.pop(j)
            t_ps = ps_t.tile([128, 1024], fp32, tag="ps_t")
            # one id16 LDWEIGHTS for both tiles, then the 4 ea stationaries
            for i in range(2):
                nc.tensor.matmul(t_ps[:, 512 * i:512 * (i + 1)], id16_t[:],
                                 xae_t[:, 512 * i:512 * (i + 1)],
                                 start=True, stop=False, skip_group_check=True)
            for i in range(2):
                for g in range(2):
                    nc.tensor.matmul(
                        t_ps[:, 512 * i + 256 * g:512 * i + 256 * (g + 1)],
                        xae_t[:, 1024 + 256 * i + 128 * g:
                              1024 + 256 * i + 128 * (g + 1)],
                        wbd_t[:], start=False, stop=(g == 1),
                        skip_group_check=True)

            # r = relu(t): 7 of 8 chunks on scalar, 1 of 8 on vector
            r_t = r_p.tile([128, 1024], fp16, tag="rr")
            if j % 8 == 7:
                nc.vector.tensor_scalar_max(r_t[:], t_ps[:], 0.0)
            else:
                nc.scalar.activation(r_t[:], t_ps[:], Act.Relu)
            # P = exp(r) (scalar); Pm = r * P (vector). One tile for both so
            # the downstream S/T matmuls carry a single semaphore wait.
            PPm = pp_p.tile([128, 2048], bf16, tag="pp")
            nc.scalar.activation(PPm[:, 0:1024], r_t[:], Act.Exp)
            nc.vector.tensor_tensor(out=PPm[:, 1024:2048], in0=r_t[:],
                                    in1=PPm[:, 0:1024], op=Alu.mult)
            chunk_pq[j] = PPm

        def emit_chunk_back(j):
            PPm = chunk_pq.pop(j)
            for i in range(2):
                T = 2 * j + i
                c = T % 4
                if c == 0:
                    s1_pair[0] = ps_s.tile([128, 512], fp32, tag="ps_s",
                                           name=f"S1_{T}")
                    s1_pair[1] = ps_u.tile([128, 512], fp32, tag="ps_u",
                                           name=f"T1_{T}")
                S1_ps, T1_ps = s1_pair
                nc.tensor.matmul(S1_ps[32 * c:32 * c + 32, :], bd_t[:],
                                 PPm[:, 512 * i:512 * (i + 1)],
                                 start=True, stop=True, tile_position=(0, 32 * c),
                                 skip_group_check=True)
                nc.tensor.matmul(T1_ps[32 * c:32 * c + 32, :], bd_t[:],
                                 PPm[:, 1024 + 512 * i:1024 + 512 * (i + 1)],
                                 start=True, stop=True, tile_position=(0, 32 * c),
                                 skip_group_check=True)

                if c == 3:
                    G = T // 4
                    nc.vector.tensor_reduce(
                        S2_all[:, G * 64:(G + 1) * 64],
                        S1_ps[:].rearrange("p (b h) -> p h b", h=H),
                        axis=mybir.AxisListType.X, op=Alu.add)
                    nc.vector.tensor_reduce(
                        T2_all[:, G * 64:(G + 1) * 64],
                        T1_ps[:].rearrange("p (b h) -> p h b", h=H),
                        axis=mybir.AxisListType.X, op=Alu.add)
                    if G % 4 == 3:
                        emit_batch_dve(G // 4)

        issue_dma(0)
        issue_dma(1)
        for j in range(nchunk):
            if j >= 10 and (j - 10) % 8 == 0:
                emit_batch_pe((j - 10) // 8)
            issue_dma(j + 2)
            emit_chunk_front(j)
            if j >= 1:
                emit_chunk_back(j - 1)
        emit_chunk_back(nchunk - 1)
        for B in sorted(pending):
            emit_batch_pe(B)

        # ---- finalize BN stats
        s1 = sb.tile([128, 1], fp32)
        nc.vector.tensor_reduce(s1[:], s1p[:], axis=mybir.AxisListType.X,
                                op=Alu.add)
        s2 = sb.tile([128, 1], fp32)
        nc.vector.tensor_reduce(s2[:], s2p[:], axis=mybir.AxisListType.X,
                                op=Alu.add)
        stats = sb.tile([128, 2], fp32)
        nc.vector.tensor_copy(stats[:, 0:1], s1[:])
        nc.vector.tensor_copy(stats[:, 1:2], s2[:])
        nc.scalar.dma_start(st_d[:], stats[:])

    _legalize_multiwaits(nc)
    return nc


# ------------------------------------------------------------ phase-2 kernel
def _build_phase2(ntiles=NT):
    fp32 = mybir.dt.float32
    bf16 = mybir.dt.bfloat16
    Act = mybir.ActivationFunctionType
    npc = ntiles * 32
    nslice = npc // 512
    nc = bass.Bass()

    h1_d = nc.declare_dram_parameter("h1", [128, npc], fp32, isOutput=False)
    # cw2: [128, 66] = [scale | shift | W2 (bf16-packed as fp32 bits? no:
    # scale/shift fp32 cols 0-1, then W2 bf16 staged separately)
    ss_d = nc.declare_dram_parameter("ss", [128, 2], fp32, isOutput=False)
    w2_d = nc.declare_dram_parameter("w2", [128, 64], bf16, isOutput=False)
    out_d = nc.declare_dram_parameter("out", [64, npc], fp32, isOutput=True)

    with tile.TileContext(nc) as tc, ExitStack() as ctx:
        const = ctx.enter_context(tc.tile_pool(name="const", bufs=1))
        h1_p = ctx.enter_context(tc.tile_pool(name="h1p", bufs=4))
        h2_p = ctx.enter_context(tc.tile_pool(name="h2p", bufs=3))
        osl_p = ctx.enter_context(tc.tile_pool(name="osl", bufs=3))
        ps = ctx.enter_context(tc.tile_pool(name="ps", bufs=3, space="PSUM"))

        ss_t = const.tile([128, 2], fp32)
        nc.sync.dma_start(ss_t[:], ss_d[:])
        w2_t = const.tile([128, 64], bf16)
        nc.sync.dma_start(w2_t[:], w2_d[:])

        pro_ps = ps.tile([128, 512], fp32, tag="ps")
        nc.tensor.matmul(pro_ps[0:8, 0:8], w2_t[:, 0:8], w2_t[:, 0:8],
                         start=True, stop=True, skip_group_check=True)

        for j in range(npc // 1024):
            h1sl = h1_p.tile([128, 1024], fp32, tag="h1p")
            nc.sync.dma_start(h1sl[:], h1_d[:, j * 1024:(j + 1) * 1024])
            h2sl = h2_p.tile([128, 1024], bf16, tag="h2p")
            nc.scalar.activation(h2sl[:], h1sl[:], Act.Relu, bias=ss_t[:, 1:2],
                                 scale=ss_t[:, 0:1])
            oslice = osl_p.tile([64, 1024], fp32, tag="osl")
            for g in range(2):
                o_ps = ps.tile([128, 512], fp32, tag="ps")
                nc.tensor.matmul(o_ps[0:64, :], w2_t[:],
                                 h2sl[:, 512 * g:512 * (g + 1)],
                                 start=True, stop=True, skip_group_check=True)
                nc.vector.tensor_copy(oslice[:, 512 * g:512 * (g + 1)],
                                      o_ps[0:64, :])
            nc.sync.dma_start(out_d[:, j * 1024:(j + 1) * 1024], oslice[:])

    _legalize_multiwaits(nc)
    return nc


# -------------------------------------------------------------- host staging
def _stage_core(x_c, xs_slot_c, ea_slot_c, ninv_c, ntiles=NT):
    """xs_slot_c: [npc, K, H] f32 (x[src], invalid slots = NEG_BIG)
    ea_slot_c: [npc, K, ED] f32;  ninv_c: [npc] f32.

    Returns (xae bf16 [128, nchunk*1536], corr f32, xres f32)."""
    a = xs_slot_c.reshape(ntiles, 32, 8, 4, H)          # [T, m, b, j, h]
    xs_dev = np.ascontiguousarray(
        a.transpose(1, 3, 0, 2, 4)).reshape(128, ntiles * 512)

    b = ea_slot_c.reshape(ntiles, 32, 8, 4, ED)         # [T, m, b, j, d]
    ea4 = np.ascontiguousarray(
        b.transpose(2, 4, 0, 1, 3)                      # [b, d, T, m, j]
        .reshape(2, 4, ED, ntiles, 128)                 # [g, r, d, T, e']
        .transpose(1, 2, 3, 0, 4)                       # [r, d, T, g, e']
    ).reshape(128, ntiles * 256)

    # fuse xs + ea into one per-chunk DMA block:
    # chunk j: [xs(2j) 512 | xs(2j+1) 512 | ea(2j) 256 | ea(2j+1) 256]
    nch = ntiles // 2
    xs3 = xs_dev.reshape(128, nch, 1024)
    ea3 = ea4.reshape(128, nch, 512)
    xae = np.concatenate([xs3, ea3], axis=2).astype(np_bf16)
    xae = np.ascontiguousarray(xae).reshape(128, nch * 1536)

    # node n = 128*G + p'  (p' = 32*(T%4) + node%32)
    corr = (ninv_c.astype(np.float32) - 1e-16)[:, None] * np.ones((1, H), np.float32)
    corr_dev = np.ascontiguousarray(
        corr.reshape(ntiles // 4, 128, H).transpose(1, 0, 2)).reshape(128, -1)
    xres_dev = np.ascontiguousarray(
        x_c.reshape(ntiles // 4, 128, H).transpose(1, 0, 2)).reshape(128, -1)
    return xae, corr_dev, xres_dev


def _consts(W_edge):
    Wbd = np.zeros((128, 256), np.float32)
    for r in range(4):
        Wbd[32 * r:32 * r + 32, 64 * r:64 * r + 64] = W_edge
    BD = np.zeros((128, 32), np.float32)
    for m in range(32):
        BD[4 * m:4 * m + 4, m] = 1.0
    ident = np.eye(128, dtype=np.float32)
    return Wbd, BD, ident


def build_in_maps(x, edge_index, edge_attr, nbr, W_edge, W1):
    """Stage the full inputs into per-core phase-1 input maps."""
    x = np.ascontiguousarray(np.asarray(x, dtype=np.float32))
    edge_attr = np.ascontiguousarray(np.asarray(edge_attr, dtype=np.float32))
    W_edge = np.ascontiguousarray(np.asarray(W_edge, dtype=np.float32))
    W1 = np.ascontiguousarray(np.asarray(W1, dtype=np.float32))

    src = np.asarray(edge_index[0], dtype=np.int64)
    nbr = np.asarray(nbr)
    valid = nbr >= 0                                    # [N, K]
    expect = np.arange(E, dtype=np.int64).reshape(N, K)
    assert np.array_equal(np.where(valid, nbr, expect), expect), \
        "kernel assumes nbr[i,k] == i*K+k on valid slots"

    src_slot = src.reshape(N, K)
    xs_slot = x[src_slot]                               # host halo: [N, K, H]
    xs_slot[~valid] = NEG_BIG
    ninv = (~valid).sum(axis=1).astype(np.float32)      # [N]
    ea_slot = edge_attr.reshape(N, K, ED)

    Wbd, BD, ident = _consts(W_edge)
    Wbd16 = Wbd.astype(np_bf16)
    BD16 = BD.astype(np_bf16)
    id16 = ident.astype(np_bf16)

    in_maps = []
    for core in range(NCORES):
        sl = slice(core * NPC, (core + 1) * NPC)
        xae, corr_dev, xres_dev = _stage_core(
            x[sl], xs_slot[sl], ea_slot[sl], ninv[sl])
        in_maps.append({
            "xae": xae, "wbd": Wbd16, "bd": BD16, "id16": id16,
            "id32": ident, "corr": corr_dev, "xres": xres_dev, "w1": W1.astype(np_bf16),
        })
    return in_maps


def kernel(x, edge_index, edge_attr, nbr, W_edge, W1, gamma, beta, W2):
    gamma = np.asarray(gamma, dtype=np.float32)
    beta = np.asarray(beta, dtype=np.float32)
    W2 = np.ascontiguousarray(np.asarray(W2, dtype=np.float32))

    in_maps = build_in_maps(x, edge_index, edge_attr, nbr, W_edge, W1)

    if "p1" not in _compiled:
        _compiled["p1"] = _build_phase1(NT)
        _compiled["p2"] = _build_phase2(NT)

    res1 = run_bass_kernel_spmd(_compiled["p1"], in_maps,
                                core_ids=list(range(NCORES)))

    # host: combine BN stats (tiny 128-vector arithmetic), build scale/shift
    s1 = np.zeros(2 * H, np.float64)
    s2 = np.zeros(2 * H, np.float64)
    for core in range(NCORES):
        st = res1.results[core]["stats"].astype(np.float64)
        s1 += st[:, 0]
        s2 += st[:, 1]
    mean = (s1 / N).astype(np.float32)
    var = (s2 / N).astype(np.float32) - mean * mean
    scale = gamma / np.sqrt(var + 1e-5)
    shift = beta - mean * scale
    ss = np.stack([scale, shift], axis=1).astype(np.float32)  # [128, 2]

    W2_16 = W2.astype(np_bf16)
    in_maps2 = [{"h1": res1.results[core]["h1"], "ss": ss, "w2": W2_16}
                for core in range(NCORES)]
    res2 = run_bass_kernel_spmd(_compiled["p2"], in_maps2,
                                core_ids=list(range(NCORES)))

    out = np.empty((N, H), np.float32)
    for core in range(NCORES):
        out[core * NPC:(core + 1) * NPC] = res2.results[core]["out"].T
    return out
